# revision 26
# baseline (speedup 1.0000x reference)
"""Multi-head causal self-attention (B=2, S=2048, D=1024, H=16) on 8 TRN2
NeuronCores via Bass/Tile.

Sharding: core c -> (batch b = c // 4, head-group g = c % 4). Each core
computes q/k/v projections for its 4 heads (256 of 1024 projection cols),
causal flash attention for those heads, and a partial output projection
(row-parallel over the head dim). Host sums the 4 partials per batch.

v4 (baseline 360us -> v3 208us -> this):
  * all matmul operands bf16 (fp32r measured 2 PE-cycles/moving-row on HW;
    bf16 is 1); K-projection bias dropped (cancels in softmax); bv/bo on host
  * HEAD-PAIR PACKED scores: the two heads of an e-block live on partition
    halves 0:64 / 64:128, so their K=64 score matmuls target different PE
    row-groups and run concurrently (tile_position auto-derived from the
    lhsT base partition) -> score time halves
  * one [128, 2, 512] score PSUM tile per k-block (slot per head); exp fused
    across both heads in one ACT instruction; diagonal blocks trimmed to the
    causally-live columns in scores/exp/mask/AV
  * softmax 1/denom = exp(-ln(denom)) on ACT (one table set, no switches),
    batched over the head pair via a shared [65, 2, 512] AV accumulator
  * flat software pipeline over (chunk, e-block) units: normalization of the
    previous unit is emitted first (frees the AV tile early), then attention,
    then prev-chunk out-projection and next-chunk projection slices as PE
    filler; engines drain their streams in priority(=emission) order
  * input DMA ordered so K-proj of chunk 0 starts after ~2.5us (wk then the
    first 512 x-columns split across both DMA queues); output DMA alternates
    between the sync HWDGE and gpsimd SWDGE queues
  * PE/ACT warmed up with dummy work during the initial DMA so the HAM
    clock-gate opens (4/8 -> 8/8) before the real matmuls

Device layouts (contraction dim on partitions everywhere):
  xT   [D, S]   : x[b].T, host-transposed
  Q^T/K^T [e, S]: head dim on partitions (2 heads per 128-partition tile)
  V    [k, e+1] : natural, with a ones column per head; the ones column turns
                  the AV^T matmul into (unnormalized AV^T, softmax denom) rows
  A^T  [e, S]   : produced directly by AV^T matmul, consumed as moving
                  operand of the output projection -> zero on-chip transposes
  outT [D, S]   : transposed partial output, host sums + transposes back
"""

from contextlib import ExitStack

import numpy as np
import ml_dtypes

import concourse.bass as bass
import concourse.mybir as mybir
import concourse.tile as tile
from concourse.bass_utils import run_bass_kernel_spmd

# Problem constants (hardcoded per harness contract).
B, S, D, NH, DH = 2, 2048, 1024, 16, 64
N_CORES = 8
GROUPS = 4                 # head-groups; cores per batch
HPC = NH // GROUPS         # heads per core = 4
E = HPC * DH               # per-core projection width = 256
P = 128                    # SBUF partitions
SC = 512                   # moving-operand chunk (q chunk)
ND = D // P                # 8 d-chunks
NEB = E // P               # 2 e-blocks per core
NQ = S // SC               # 4 q chunks
NKB = S // P               # 16 k blocks
SCALE = DH ** -0.5

F32 = mybir.dt.float32
F16 = mybir.dt.float16
BF16 = mybir.dt.bfloat16


def _split_multiwait(nc, max_waits=1):
    """This toolchain's walrus codegen accepts at most one sync-wait per
    instruction ("Too many sync wait commands"). Tile emits multi-wait
    instructions (notably the kernel-tail Drain). Keep the last wait (+ all
    updates) on the original instruction and hoist earlier waits onto
    single-wait Drains inserted before it on the same engine."""
    for f in nc.m.functions:
        for bb in f.blocks:
            new = []
            changed = False
            for inst in bb.instructions:
                si = inst.sync_info
                waits = list(si.on_wait) if si is not None and si.on_wait else []
                if len(waits) > max_waits:
                    for j, w in enumerate(waits[:-max_waits]):
                        d = mybir.InstDrain(name=f"{inst.name}-sw{j}", ins=[], outs=[])
                        d.engine = inst.engine
                        d.sync_info = mybir.SyncInfo(on_wait=[w], on_update=[])
                        new.append(d)
                    inst.sync_info = mybir.SyncInfo(
                        on_wait=waits[-max_waits:],
                        on_update=list(si.on_update) if si.on_update else [],
                    )
                    changed = True
                new.append(inst)
            if changed:
                bb.instructions = new


def build_nc():
    nc = bass.Bass("TRN2", target_bir_lowering=False, debug=False,
                   num_devices=N_CORES)

    xT = nc.dram_tensor("xT", [D, S], BF16, kind="ExternalInput")
    wqT = nc.dram_tensor("wqT", [D, E], BF16, kind="ExternalInput")
    wkT = nc.dram_tensor("wkT", [D, E], BF16, kind="ExternalInput")
    wvT = nc.dram_tensor("wvT", [D, E], BF16, kind="ExternalInput")
    woT = nc.dram_tensor("woT", [E, D], BF16, kind="ExternalInput")
    bq = nc.dram_tensor("bq", [E], F32, kind="ExternalInput")
    mk2d = nc.dram_tensor("mk2d", [P, NQ * 2 * SC], BF16, kind="ExternalInput")
    outT = nc.dram_tensor("outT", [D, S], F16, kind="ExternalOutput")

    AF = mybir.ActivationFunctionType
    with tile.TileContext(nc) as tc:
        with ExitStack() as ctx:
            const = ctx.enter_context(tc.tile_pool(name="const", bufs=1))

            # ---- persistent SBUF tensors (chunked for fine-grained deps) ----
            # x and weights live in single consolidated tiles (d-chunks on
            # a free dim) so each input tensor needs ONE HWDGE descriptor
            # per 512-column chunk -- the SWDGE queue measured far too slow
            # for the startup-critical loads
            x_sb = const.tile([P, ND, S], BF16, tag="x", name="x")
            wq_sb = const.tile([P, ND, E], BF16, tag="wq", name="wq")
            wk_sb = const.tile([P, ND, E], BF16, tag="wk", name="wk")
            wv_sb = const.tile([P, ND, E], BF16, tag="wv", name="wv")
            wo_sb = const.tile([P, NEB, D], BF16, tag="wo", name="wo")
            bq_sb = const.tile([P, NEB], F32, tag="bq", name="bq")
            qts = [[const.tile([P, SC], BF16, tag=f"qt{e}{c}", name=f"qt{e}{c}") for c in range(NQ)]
                   for e in range(NEB)]
            kts = [[const.tile([P, SC], BF16, tag=f"kt{e}{c}", name=f"kt{e}{c}") for c in range(NQ)]
                   for e in range(NEB)]
            v_sbs = [const.tile([P, NQ, HPC * (DH + 1)], BF16, tag=f"v{i}", name=f"v{i}")
                     for i in range(NQ)]
            at_sbs = [[const.tile([P, SC], BF16, tag=f"at{i}{f}", name=f"at{i}{f}")
                       for f in range(NEB)] for i in range(NQ)]
            # mk2[m][b2][kk, qq]: causal mask for diagonal-offset m, replicated
            # in the middle dim so one TT covers both heads of a pair
            mk2_sb = const.tile([P, NQ, 2, SC], BF16, tag="mk2", name="mk2")
            ones_sb = const.tile([1, DH], BF16, tag="ones", name="ones")
            warm_sb = const.tile([P, SC], BF16, tag="warm", name="warm")

            # ---- input DMAs: all on the sync HWDGE queue (one descriptor
            # per tensor/chunk), ordered so chunk-0 K-projection starts
            # ~4us in and each later x chunk lands well before its use ----
            xTr = xT.rearrange("(nd p) s -> p nd s", p=P)
            # parallel doorbells: the startup-critical tensors trigger from
            # different engine queues so descriptor generation overlaps
            nc.sync.dma_start(wk_sb[:], wkT.rearrange("(nd p) e -> p nd e", p=P))
            nc.sync.dma_start(x_sb[:, :, 0:SC], xTr[:, :, 0:SC])
            nc.sync.dma_start(wv_sb[:], wvT.rearrange("(nd p) e -> p nd e", p=P))
            nc.sync.dma_start(wq_sb[:], wqT.rearrange("(nd p) e -> p nd e", p=P))
            nc.sync.dma_start(x_sb[:, :, SC:2 * SC], xTr[:, :, SC:2 * SC])
            for cc in range(2, NQ):
                nc.sync.dma_start(x_sb[:, :, cc * SC:(cc + 1) * SC],
                                  xTr[:, :, cc * SC:(cc + 1) * SC])
            nc.sync.dma_start(wo_sb[:], woT.rearrange("(f p) d -> p f d", p=P))
            nc.sync.dma_start(bq_sb[:], bq.rearrange("(n p) -> p n", p=P))
            nc.scalar.dma_start(mk2_sb[:], mk2d[:, :])

            tmp = ctx.enter_context(tc.tile_pool(name="tmp", bufs=1))

            # PSUM pools (8 banks): psq = 2-bank score tiles x2, pav = one
            # 2-bank AV accumulator (slot per head), pgen = shared 1-bank
            # ring for projections / out-proj / recip-broadcast / warmup.
            psq = ctx.enter_context(tc.tile_pool(name="psq", bufs=2, space="PSUM"))
            pav = ctx.enter_context(tc.tile_pool(name="pav", bufs=1, space="PSUM"))
            pgen = ctx.enter_context(tc.tile_pool(name="pgen", bufs=2, space="PSUM"))
            ptp = ctx.enter_context(tc.tile_pool(name="ptp", bufs=4))
            rcp = ctx.enter_context(tc.tile_pool(name="rcp", bufs=2))
            obp = ctx.enter_context(tc.tile_pool(name="obp", bufs=4))

            # ---- engine warmup, first thing: dummy matmuls open the HAM
            # clock-gate during the initial DMA wait; dummy exp/ln pull the
            # ACT table load off the critical path ----
            warm_ps = pgen.tile([P, SC], F32, tag="g", name="gps")
            nc.gpsimd.memset(warm_sb[:], 0.0)
            for i in range(10):
                nc.tensor.matmul(warm_ps[:], lhsT=warm_sb[:, :P], rhs=warm_sb[:],
                                 start=True, stop=True)
            one_f32 = tmp.tile([P, 1], F32, tag="onef", name="onef")
            nc.vector.memset(one_f32[:], 1.0)
            nc.vector.tensor_copy(ones_sb[:],
                                  one_f32[0:1, 0:1].broadcast_to([1, DH]))
            warm_act = tmp.tile([P, 8], F32, tag="wact", name="warmact")
            nc.scalar.activation(warm_act[:], one_f32[:].broadcast_to([P, 8]),
                                 AF.Exp, scale=1.0)
            nc.scalar.activation(warm_act[:], one_f32[:].broadcast_to([P, 8]),
                                 AF.Ln)

            def build_constants():
                """v ones-columns; the causal mask now arrives prebuilt via
                DMA so the early DVE queue carries only projection
                evictions (a 6.5us startup stall traced to the Q eviction
                sitting behind the on-device mask-cast chain)."""
                for cc in range(NQ):
                    nc.vector.tensor_copy(
                        v_sbs[cc][:, :, DH::DH + 1],
                        one_f32[:, :, None].broadcast_to([P, NQ, HPC]))

            def keep_warm(n):
                wt = pgen.tile([P, SC], F32, tag="g", name="gps")
                for _ in range(n):
                    nc.tensor.matmul(wt[:], lhsT=warm_sb[:, :P], rhs=warm_sb[:],
                                     start=True, stop=True)

            def project_qk(w_sb, o_tiles, c, bias, eb):
                ps = pgen.tile([P, SC], F32, tag="g", name="gps")
                for di in range(ND):
                    nc.tensor.matmul(
                        ps[:],
                        lhsT=w_sb[:, di, eb * P:(eb + 1) * P],
                        rhs=x_sb[:, di, c * SC:(c + 1) * SC],
                        start=(di == 0), stop=(di == ND - 1),
                    )
                if bias:
                    nc.vector.tensor_scalar_add(
                        out=o_tiles[eb][c][:], in0=ps[:],
                        scalar1=bq_sb[:, eb:eb + 1])
                else:
                    nc.vector.tensor_copy(o_tiles[eb][c][:], ps[:])

            def project_v(c, kk):
                kb = c * NQ + kk
                ps = pgen.tile([P, SC], F32, tag="g", name="gps")
                for di in range(ND):
                    nc.tensor.matmul(
                        ps[:, :E],
                        lhsT=x_sb[:, di, kb * P:(kb + 1) * P],
                        rhs=wv_sb[:, di, :],
                        start=(di == 0), stop=(di == ND - 1),
                    )
                dst = v_sbs[c][:, kk, :].rearrange(
                    "p (h e) -> p h e", h=HPC)[:, :, :DH]
                nc.vector.tensor_copy(
                    dst, ps[:, :E].rearrange("p (h e) -> p h e", h=HPC))

            def proj_slice(c, et):
                if et == 0:
                    project_qk(wk_sb, kts, c, False, 0)
                    project_qk(wk_sb, kts, c, False, 1)
                    project_qk(wq_sb, qts, c, True, 0)
                    project_qk(wq_sb, qts, c, True, 1)
                else:
                    for kk in range(NQ):
                        project_v(c, kk)

            avs = {}
            pending = {}

            def scores_exp(c, et, j):
                """One k-block: head-pair packed score matmuls + fused exp."""
                nkb_c = NQ * (c + 1)
                m = j - NQ * c
                q0 = P * m if m > 0 else 0
                ps = psq.tile([P, 2, SC], F32, tag="sq", name="sq")
                for b2 in range(2):
                    er = b2 * DH
                    nc.tensor.matmul(
                        ps[:, b2, q0:],
                        lhsT=kts[et][j // NQ][er:er + DH,
                                              (j % NQ) * P:(j % NQ + 1) * P],
                        rhs=qts[et][c][er:er + DH, q0:],
                        start=True, stop=True,
                    )
                pt = ptp.tile([P, 2, SC], BF16, tag="pt", name="pt")
                pending[(c, et, j)] = pt
                nc.scalar.activation(pt[:, :, q0:], ps[:, :, q0:], AF.Exp,
                                     scale=SCALE)

            def finish(c, et, j):
                """Mask (diagonal) + AV accumulation for one k-block."""
                av = avs[(c, et)]
                nkb_c = NQ * (c + 1)
                pt = pending.pop((c, et, j))
                m = j - NQ * c
                q0 = P * m if m > 0 else 0
                if m >= 0:
                    nc.vector.tensor_mul(pt[:, :, q0:], pt[:, :, q0:],
                                         mk2_sb[:, m, :, q0:])
                for b2 in range(2):
                    h = 2 * et + b2
                    nc.tensor.matmul(
                        av[:, b2, q0:],
                        lhsT=v_sbs[j // NQ][:, j % NQ,
                                            h * (DH + 1):(h + 1) * (DH + 1)],
                        rhs=pt[:, b2, q0:],
                        start=(j == 0), stop=(j == nkb_c - 1),
                    )

            def prologue(c, et):
                """First two k-blocks' scores+exp, emitted early so ACT has
                work while PE drains the previous unit's filler."""
                scores_exp(c, et, 0)
                scores_exp(c, et, 1)

            def body(c, et):
                av = pav.tile([DH + 1, 2, SC], F32, tag="av", name="av")
                avs[(c, et)] = av
                nkb_c = NQ * (c + 1)
                for j in range(2, nkb_c):
                    scores_exp(c, et, j)
                    finish(c, et, j - 2)
                finish(c, et, nkb_c - 2)
                finish(c, et, nkb_c - 1)

            def normalize(c, et):
                """1/denom = exp(-ln(denom)) on ACT, batched over the head
                pair; broadcast over the head dim via PE outer products."""
                av = avs.pop((c, et))
                ln2 = rcp.tile([1, 2, SC], F32, tag="ln", name="ln")
                rc2 = rcp.tile([1, 2, SC], BF16, tag="rc", name="rc")
                nc.scalar.activation(ln2[0:1, :, :], av[DH:DH + 1, :, :], AF.Ln)
                with nc.allow_low_precision(
                        reason="bf16 softmax recip rounding is benign"):
                    nc.scalar.activation(rc2[0:1, :, :], ln2[0:1, :, :], AF.Exp,
                                         scale=-1.0)
                for b2 in range(2):
                    rb = pgen.tile([DH, SC], F32, tag="g", name="gps")
                    nc.tensor.matmul(rb[:], lhsT=ones_sb[0:1, :],
                                     rhs=rc2[0:1, b2, :], start=True, stop=True)
                    rcb = rcp.tile([DH, SC], F32, tag="rcb", name="rcb")
                    nc.vector.tensor_copy(rcb[:], rb[:])
                    er = b2 * DH
                    nc.vector.tensor_mul(
                        at_sbs[c][et][er:er + DH, :], av[0:DH, b2, :], rcb[:])

            def out_proj_eb(c, eb):
                po = pgen.tile([P, SC], F32, tag="g", name="gps")
                for ft in range(NEB):
                    nc.tensor.matmul(
                        po[:],
                        lhsT=wo_sb[:, ft, eb * P:(eb + 1) * P],
                        rhs=at_sbs[c][ft][:],
                        start=(ft == 0), stop=(ft == NEB - 1),
                    )
                ob = obp.tile([P, SC], F16, tag="ob", name="ob")
                nc.vector.tensor_copy(ob[:], po[:])
                nc.sync.dma_start(
                    outT[eb * P:(eb + 1) * P, c * SC:(c + 1) * SC], ob[:])

            def out_proj_group(c, ebs):
                for eb in ebs:
                    out_proj_eb(c, eb)

            # ---- flat pipeline over (chunk, e-block) units ----
            # Emission order IS dependency order in Tile, so every producer
            # precedes its consumers; within that constraint, the next
            # unit's first two score blocks (prologue) are emitted before
            # the current unit's filler so ACT never starves at unit
            # boundaries, and filler (prev-chunk out-proj, next-chunk
            # projections) sits last in priority.
            def filler_quanta(c, et):
                qs = []
                if c > 0:
                    for k in range(4):
                        qs.append(lambda eb=4 * et + k: out_proj_eb(c - 1, eb))
                    qs.append(lambda: keep_warm(2))
                if c + 1 < NQ:
                    if et == 0:
                        qs.append(lambda: project_qk(wk_sb, kts, c + 1, False, 0))
                        qs.append(lambda: project_qk(wk_sb, kts, c + 1, False, 1))
                        qs.append(lambda: project_qk(wq_sb, qts, c + 1, True, 0))
                        qs.append(lambda: project_qk(wq_sb, qts, c + 1, True, 1))
                    else:
                        for kk in range(NQ):
                            qs.append(lambda kk=kk: project_v(c + 1, kk))
                return qs

            proj_slice(0, 0)          # K/Q of chunk 0
            prologue(0, 0)
            build_constants()
            proj_slice(0, 1)          # V of chunk 0 (before body needs it)
            units = [(c, et) for c in range(NQ) for et in range(NEB)]
            for i, (c, et) in enumerate(units):
                if i > 0:
                    normalize(*units[i - 1])
                    keep_warm(3)
                body(c, et)
                if i + 1 < len(units):
                    prologue(*units[i + 1])
                for q in filler_quanta(c, et):
                    q()
            normalize(*units[-1])
            # bridge the normalize->out-proj dependency gap with dummy
            # matmuls so the PE clock-gate stays open through the tail
            keep_warm(6)
            for eb in range(D // P):
                out_proj_eb(NQ - 1, eb)
                if eb % 3 == 2:
                    keep_warm(2)

    _split_multiwait(nc)
    return nc


_NC_CACHE = None
_last_in_maps = None


def kernel(**inputs):
    global _NC_CACHE, _last_in_maps
    if _NC_CACHE is None:
        _NC_CACHE = build_nc()
    nc = _NC_CACHE

    bf = ml_dtypes.bfloat16
    x = np.asarray(inputs["x"], np.float32)
    Wq = np.asarray(inputs["Wq"], np.float32)
    Wk = np.asarray(inputs["Wk"], np.float32)
    Wv = np.asarray(inputs["Wv"], np.float32)
    Wo = np.asarray(inputs["Wo"], np.float32)
    bq = np.asarray(inputs["bq"], np.float32)
    bv = np.asarray(inputs["bv"], np.float32)
    bo = np.asarray(inputs["bo"], np.float32)
    # The mask input is causal (tril ones) by construction; the kernel
    # hardcodes causal structure. bk is dropped: a k-side bias adds the
    # same offset to every score within a softmax row, so it cancels.

    kk = np.arange(P)[:, None]
    qq = np.arange(SC)[None, :]
    mk2_host = np.stack([(kk + P * m <= qq) for m in range(NQ)], 0)
    mk2_host = np.broadcast_to(mk2_host[:, None, :, :], (NQ, 2, P, SC))
    mk2_host = np.ascontiguousarray(
        mk2_host.transpose(2, 0, 1, 3).reshape(P, NQ * 2 * SC).astype(bf))

    xTs = [np.ascontiguousarray(x[b].T.astype(bf)) for b in range(B)]
    in_maps = []
    for c in range(N_CORES):
        b, g = divmod(c, GROUPS)
        rows = slice(g * E, (g + 1) * E)
        in_maps.append({
            "xT": xTs[b],
            "wqT": np.ascontiguousarray(Wq[rows].T.astype(bf)),
            "wkT": np.ascontiguousarray(Wk[rows].T.astype(bf)),
            "wvT": np.ascontiguousarray(Wv[rows].T.astype(bf)),
            "woT": np.ascontiguousarray(Wo[:, rows].T.astype(bf)),
            "bq": np.ascontiguousarray(bq[rows]),
            "mk2d": mk2_host,
        })

    _last_in_maps = in_maps
    res = run_bass_kernel_spmd(nc, in_maps, list(range(N_CORES)))

    out = np.zeros((B, S, D), np.float32)
    for c in range(N_CORES):
        b = c // GROUPS
        out[b] += res.results[c]["outT"].T.astype(np.float32)
    # bv enters only additively after softmax (rows of P sum to 1):
    # out += Wo @ bv; plus the output bias bo.
    out += (Wo @ bv + bo)[None, None, :]
    return out
